# revision 21
# baseline (speedup 1.0000x reference)
"""GCN layer (gather + scatter-add message passing) on 8 Trainium2 NeuronCores.

Strategy (dst-partitioned node sharding, per the sharding hint):
  - Node blocks of 128; block b is owned by core b % 8, slot b // 8.
  - Host sorts edges by (dst block, src-table-half), appends self-loops, and
    pads each (block, half) edge group to a multiple of 128.  All floating
    point math happens on device.
  - Device phase A: every core builds the gather table yp = dinv * x in fp16
    in its own HBM (dinv = 1/sqrt(deg), deg computed on device from the
    per-node edge counts that fall out of the host-side sort).
  - Device phase B: per 128-edge tile, dma_gather 256B rows of yp by src,
    build a one-hot selection matrix S[e, n] = (dstloc[e] == n) on the vector
    engine, and matmul-accumulate psum[f, n] += msg[e, f]^T @ S[e, n] on the
    tensor engine.  The dst-sorted layout means each 128-node block
    accumulates entirely in PSUM - no scatter to HBM at all.
  - Per block: out[n, o] = dinv[n] * (agg^T @ W)[n, o] + b[o], DMA'd straight
    to the core's output slice.  (The linear layer commutes with the
    aggregation, so the GEMM runs on the 6.3k aggregated rows per core
    instead of all 50k input rows.)

The edge tables are padded so the instruction stream is identical on all 8
cores (run_bass_kernel_spmd compiles one program); only tensor data differs.
"""

import sys

sys.path.insert(0, "/opt/trn_rl_repo")

import numpy as np

import concourse.bass as bass
import concourse.bacc as bacc
import concourse.mybir as mybir
import concourse.tile as tile
import concourse.tile_sem_assignment as _tsa
from concourse.tile import add_dep_helper

# Tile round-robins SWDGE DMAs over the 8 DMASW sem lanes in scheduling
# order, which lets one sem serve instructions on different SWDGE queues.
# The ucode's per-queue ring reclaim then sees foreign increments (CoreSim
# flags this as "sem locked to SWDGE queue").  Pin lanes per queue instead:
# queue q only ever uses lanes {q, q+4}.
if not getattr(_tsa.TileClockTick, "_gcn_queue_aware", False):
    _orig_assign_tick = _tsa.TileClockTick._assign_tick

    def _assign_tick_queue_aware(self, inst):
        if (
            isinstance(inst, _tsa.DMAInst)
            and inst.engine == mybir.EngineType.Pool
            and not isinstance(inst, _tsa.bass_isa.UserSyncedRemoteDMADescs)
            and self.swdge_sem_count == _tsa.NUM_SWDGE_GLOBAL_SEMS
        ):
            q = getattr(inst, "queue_num", 0) or 0
            toggles = getattr(self, "_gcn_q_toggle", None)
            if toggles is None:
                toggles = self._gcn_q_toggle = [0, 0, 0, 0]
            self.next_sw_dma_idx = q + 4 * toggles[q]
            toggles[q] ^= 1
        return _orig_assign_tick(self, inst)

    _tsa.TileClockTick._assign_tick = _assign_tick_queue_aware
    _tsa.TileClockTick._gcn_queue_aware = True

N = 50000
E = 800000
F = 128          # in/out channels
P = 128
NCORES = 8
NB = 392         # node blocks incl. padding (= 8 * 49)
G = NB // NCORES  # 49 slots per core
LO = 32768       # gather-table split (int16 index limit)
NPAD = 51200     # padded node rows (= 25 chunks of 2048)
CHUNK = 2048
NCHUNKS = NPAD // CHUNK
QN = CHUNK // P  # node blocks per phase-A chunk
SPQ = 7          # slots per gather chunk
NQ = G // SPQ    # 7 gather chunks

f32 = mybir.dt.float32
fp16 = mybir.dt.float16
i32 = mybir.dt.int32
i16 = mybir.dt.int16


def _host_prep(x, W, b, edge_index):
    """Pure index manipulation + data staging; no FP math on the inputs."""
    x = np.asarray(x, dtype=np.float32)
    W = np.asarray(W, dtype=np.float32)
    b = np.asarray(b, dtype=np.float32)
    ei = np.asarray(edge_index)
    src = ei[0].astype(np.int64)
    dst = ei[1].astype(np.int64)

    cnt = np.bincount(dst, minlength=NPAD).astype(np.int64)

    # Sort edges by (dst block, src table half).
    ishi = (src >= LO).astype(np.int64)
    blk = dst >> 7
    order = np.lexsort((ishi, blk))
    src_s, dst_s, ishi_s, blk_s = src[order], dst[order], ishi[order], blk[order]
    bounds = np.searchsorted(blk_s, np.arange(NB + 1))

    # Per (core, slot) edge lists.  block = 8*g + c.
    lo_idx = [[None] * G for _ in range(NCORES)]
    lo_dst = [[None] * G for _ in range(NCORES)]
    hi_idx = [[None] * G for _ in range(NCORES)]
    hi_dst = [[None] * G for _ in range(NCORES)]
    for g in range(G):
        for c in range(NCORES):
            bb_ = 8 * g + c
            s0, s1 = bounds[bb_], bounds[bb_ + 1]
            mid = s0 + int(np.searchsorted(ishi_s[s0:s1], 1))
            sl = np.arange(128 * bb_, min(128 * (bb_ + 1), N), dtype=np.int64)
            li, ld = src_s[s0:mid], dst_s[s0:mid] - 128 * bb_
            hi, hd = src_s[mid:s1] - LO, dst_s[mid:s1] - 128 * bb_
            if bb_ < LO // 128:  # self loops go in the lo half
                li = np.concatenate([li, sl])
                ld = np.concatenate([ld, sl - 128 * bb_])
            else:
                hi = np.concatenate([hi, sl - LO])
                hd = np.concatenate([hd, sl - 128 * bb_])
            lo_idx[c][g], lo_dst[c][g] = li, ld
            hi_idx[c][g], hi_dst[c][g] = hi, hd

    # Shared tile counts (max over cores) keep the instruction stream uniform.
    T_LO = [max(1, max(-(-len(lo_idx[c][g]) // 128) for c in range(NCORES)))
            for g in range(G)]
    T_HI = [max(-(-len(hi_idx[c][g]) // 128) for c in range(NCORES))
            for g in range(G)]
    NT = sum(T_LO) + sum(T_HI)
    LTOT = NT * 8

    # The gather table is fp16; staging x as fp16 is numerically equivalent
    # (one extra rounding) and halves the phase-A HBM read.
    x_pad = np.zeros((NPAD, F), np.float16)
    x_pad[:N] = x.astype(np.float16)
    cnt_all = cnt.reshape(NPAD // P, P).T.astype(np.int32).copy()  # [128, 400]
    bb_host = np.tile(b[None, :], (P, 1)).astype(np.float32)
    iota_host = np.tile(np.arange(P, dtype=np.float16)[None, :], (P, 1)).copy()

    in_maps = []
    for c in range(NCORES):
        dstloc = np.full((P, NT), -1.0, np.float16)
        idx16 = np.zeros((P, LTOT), np.int16)
        # Packing order mirrors the device's gather issue order: all hi
        # segments first (issued early, while the lo table half builds),
        # then the lo segments.
        col = icol = 0
        for lists_i, lists_d, T in ((hi_idx, hi_dst, T_HI), (lo_idx, lo_dst, T_LO)):
            for g in range(G):
                nt = T[g]
                if nt == 0:
                    continue
                li, ld = lists_i[c][g], lists_d[c][g]
                pi = np.zeros(nt * 128, np.int64)
                pi[: len(li)] = li
                pd = np.full(nt * 128, -1.0, np.float32)
                pd[: len(ld)] = ld
                dstloc[:, col : col + nt] = pd.reshape(nt, 128).T
                col += nt
                k8 = nt * 8
                idx16[:, icol : icol + k8] = np.tile(
                    pi.reshape(-1, 16).T.astype(np.int16), (8, 1)
                )
                icol += k8
        cnt_slot = cnt_all[:, [8 * g + c for g in range(G)]].copy()
        in_maps.append(
            {
                "x": x_pad,
                "cnt_all": cnt_all,
                "cnt_slot": cnt_slot,
                "w": W,
                "bb": bb_host,
                "iota": iota_host,
                "dstloc": dstloc,
                "idx16": idx16,
            }
        )
    return in_maps, T_LO, T_HI


def build_nc(T_LO, T_HI, debug=False):
    NT = sum(T_LO) + sum(T_HI)
    LTOT = NT * 8
    nc = bacc.Bacc(
        "TRN2", target_bir_lowering=False, debug=debug, num_swdge_queues=4
    )

    x_d = nc.dram_tensor("x", [NPAD, F], fp16, kind="ExternalInput")
    cnt_all_d = nc.dram_tensor("cnt_all", [P, NPAD // P], i32, kind="ExternalInput")
    cnt_slot_d = nc.dram_tensor("cnt_slot", [P, G], i32, kind="ExternalInput")
    w_d = nc.dram_tensor("w", [F, F], f32, kind="ExternalInput")
    bb_d = nc.dram_tensor("bb", [P, F], f32, kind="ExternalInput")
    iota_d = nc.dram_tensor("iota", [P, P], fp16, kind="ExternalInput")
    dstloc_d = nc.dram_tensor("dstloc", [P, NT], fp16, kind="ExternalInput")
    idx_d = nc.dram_tensor("idx16", [P, LTOT], i16, kind="ExternalInput")
    out_d = nc.dram_tensor("out", [G * P, F], f32, kind="ExternalOutput")
    yp_d = nc.dram_tensor("yp", [NPAD, F], fp16)

    with tile.TileContext(nc) as tc:
        with (
            tc.tile_pool(name="const", bufs=1) as cp,
            tc.tile_pool(name="pa", bufs=3) as pa,
            tc.tile_pool(name="mlo", bufs=4) as plo,
            tc.tile_pool(name="mhi", bufs=1) as phi,  # one slot per per-slot tag
            tc.tile_pool(name="sel", bufs=5) as psel,
            tc.tile_pool(name="tt", bufs=3) as ptt,
            tc.tile_pool(name="osb", bufs=3) as posb,
            tc.tile_pool(name="agg", bufs=3, space="PSUM") as pagg,
            tc.tile_pool(name="gem", bufs=2, space="PSUM") as pgem,
        ):
            w_sb = cp.tile([F, F], f32)
            nc.sync.dma_start(out=w_sb[:], in_=w_d[:])
            bb_sb = cp.tile([P, F], f32)
            nc.sync.dma_start(out=bb_sb[:], in_=bb_d[:])
            iota_sb = cp.tile([P, P], fp16)
            nc.sync.dma_start(out=iota_sb[:], in_=iota_d[:])
            dstloc_sb = cp.tile([P, NT], fp16)
            nc.sync.dma_start(out=dstloc_sb[:], in_=dstloc_d[:])
            idx_sb = cp.tile([P, LTOT], i16)
            nc.sync.dma_start(out=idx_sb[:], in_=idx_d[:])

            # dinv = 1/sqrt(cnt + 1): full node grid + per-core slot grid
            def make_dinv(src_dram, ncols):
                t_i = cp.tile([P, ncols], i32)
                nc.sync.dma_start(out=t_i[:], in_=src_dram[:])
                t_f = cp.tile([P, ncols], f32)
                nc.vector.tensor_copy(out=t_f[:], in_=t_i[:])
                nc.scalar.activation(
                    out=t_f[:], in_=t_f[:],
                    func=mybir.ActivationFunctionType.Sqrt, bias=1.0, scale=1.0,
                )
                t_r = cp.tile([P, ncols], f32)
                nc.vector.reciprocal(out=t_r[:], in_=t_f[:])
                return t_r

            dinv_all = make_dinv(cnt_all_d, NPAD // P)
            dinv_slot = make_dinv(cnt_slot_d, G)

            # ---- phase A: yp = dinv * x, cast to fp16 ----
            # chunk k viewed as [p, q, f]: node = k*CHUNK + q*P + p
            def chunk_ap(dram, k):
                a = dram[:]
                return bass.AP(a.tensor, k * CHUNK * F, [[F, P], [P * F, QN], [1, F]])

            # hi-half chunks first: the hi gathers (35% of descriptor work)
            # can then start while the lo table half is still being built.
            n_lo_chunks = LO // CHUNK
            y_writes = [None] * NCHUNKS
            for k in list(range(n_lo_chunks, NCHUNKS)) + list(range(n_lo_chunks)):
                xt = pa.tile([P, CHUNK], fp16, tag="xt")
                nc.sync.dma_start(out=xt[:], in_=chunk_ap(x_d, k))
                yt = pa.tile([P, CHUNK], fp16, tag="yt")
                nc.vector.tensor_tensor(
                    out=yt[:].rearrange("p (q f) -> p q f", f=F),
                    in0=xt[:].rearrange("p (q f) -> p q f", f=F),
                    in1=dinv_all[:, QN * k : QN * (k + 1)].to_broadcast([P, QN, F]),
                    op=mybir.AluOpType.mult,
                )
                y_writes[k] = nc.sync.dma_start(out=chunk_ap(yp_d, k), in_=yt[:])

            # ---- phase B ----
            # One gather per (slot, table-half): independent destination
            # tiles let the 4 SWDGE queues generate descriptors concurrently
            # (HW-probed ~4x; same-tile slices would serialize under Tile).
            lo_tab = yp_d[0:LO, :]
            hi_tab = yp_d[LO:NPAD, :]
            lo_deps = y_writes[:n_lo_chunks]
            hi_deps = y_writes[n_lo_chunks:]
            col = icol = 0
            qrr = 0

            def gather(pool, tag, tab, nt, deps):
                nonlocal icol, qrr
                m = pool.tile([P, nt * F], fp16, tag=tag)
                gi = nc.gpsimd.dma_gather(
                    out_ap=m[:].rearrange("p (k f) -> p k f", f=F),
                    in_ap=tab,
                    idxs_ap=idx_sb[:, icol : icol + nt * 8],
                    num_idxs=nt * P,
                    num_idxs_reg=nt * P,
                    elem_size=F,
                    single_packet=False,
                    queue_num=qrr % 4,
                )
                qrr += 1
                for yw in deps:
                    add_dep_helper(gi.ins, yw.ins)
                icol += nt * 8
                return m

            # all hi gathers up-front (each keeps its tile until consumed)
            mhi_tiles = [None] * G
            hi_cols = [0] * G
            for g in range(G):
                if T_HI[g]:
                    mhi_tiles[g] = gather(phi, f"mhi{g}", hi_tab, T_HI[g], hi_deps)
                    hi_cols[g] = col
                    col += T_HI[g]

            for g in range(G):
                ntl, nth = T_LO[g], T_HI[g]
                ntot = ntl + nth
                mlo = None
                mhi = mhi_tiles[g]
                lo_col, hi_col = 0, hi_cols[g]
                if ntl:
                    mlo = gather(plo, "mlo", lo_tab, ntl, lo_deps)
                    lo_col = col
                    col += ntl

                agg = pagg.tile([P, P], f32, tag="agg")
                mm = 0
                for (nt, m, base_col) in ((ntl, mlo, lo_col), (nth, mhi, hi_col)):
                    if nt == 0:
                        continue
                    S = psel.tile([P, nt * P], fp16, tag="S")
                    dl = dstloc_sb[:, base_col : base_col + nt]
                    nc.vector.tensor_tensor(
                        out=S[:].rearrange("p (t j) -> p t j", j=P),
                        in0=dl.to_broadcast([P, nt, P]),
                        in1=bass.AP(
                            iota_sb[:].tensor,
                            iota_sb[:].offset,
                            [iota_sb[:].ap[0], [0, nt], [1, P]],
                        ),
                        op=mybir.AluOpType.is_equal,
                    )
                    for t in range(nt):
                        nc.tensor.matmul(
                            out=agg[:],
                            lhsT=m[:, t * F : (t + 1) * F],
                            rhs=S[:, t * P : (t + 1) * P],
                            start=(mm == 0),
                            stop=(mm == ntot - 1),
                        )
                        mm += 1

                if True:
                    tt = ptt.tile([P, P], f32, tag="tt")
                    nc.scalar.activation(
                        out=tt[:], in_=agg[:],
                        func=mybir.ActivationFunctionType.Copy,
                    )
                    gem = pgem.tile([P, P], f32, tag="gem")
                    nc.tensor.matmul(
                        out=gem[:], lhsT=tt[:], rhs=w_sb[:], start=True, stop=True
                    )
                    osb = posb.tile([P, P], f32, tag="osb")
                    nc.vector.tensor_scalar(
                        out=osb[:], in0=gem[:],
                        scalar1=dinv_slot[:, g : g + 1], scalar2=None,
                        op0=mybir.AluOpType.mult,
                    )
                    nc.vector.tensor_tensor(
                        out=osb[:], in0=osb[:], in1=bb_sb[:],
                        op=mybir.AluOpType.add,
                    )
                    nc.sync.dma_start(
                        out=out_d[g * P : (g + 1) * P, :], in_=osb[:]
                    )

    nc.compile()
    return nc


def _assemble(results):
    out = np.zeros((NB * P, F), np.float32)
    for c in range(NCORES):
        oc = results[c]["out"]
        for g in range(G):
            out[(8 * g + c) * P : (8 * g + c + 1) * P] = oc[g * P : (g + 1) * P]
    return out[:N]


def kernel(x, W, b, edge_index):
    from concourse.bass_utils import run_bass_kernel_spmd

    in_maps, T_LO, T_HI = _host_prep(x, W, b, edge_index)
    nc = build_nc(T_LO, T_HI)
    res = run_bass_kernel_spmd(nc, in_maps, list(range(NCORES)))
    return _assemble(res.results)


# revision 24
# speedup vs baseline: 1.0290x; 1.0290x over previous
"""GCN layer (gather + scatter-add message passing) on 8 Trainium2 NeuronCores.

Strategy (dst-partitioned node sharding, per the sharding hint):
  - Node blocks of 128; block b is owned by core b % 8, slot b // 8.
  - Host sorts edges by (dst block, src-table-half), appends self-loops, and
    pads each (block, half) edge group to a multiple of 128.  All floating
    point math happens on device.
  - Device phase A: every core builds the gather table yp = dinv * x in fp16
    in its own HBM (dinv = 1/sqrt(deg), deg computed on device from the
    per-node edge counts that fall out of the host-side sort).
  - Device phase B: per 128-edge tile, dma_gather 256B rows of yp by src,
    build a one-hot selection matrix S[e, n] = (dstloc[e] == n) on the vector
    engine, and matmul-accumulate psum[f, n] += msg[e, f]^T @ S[e, n] on the
    tensor engine.  The dst-sorted layout means each 128-node block
    accumulates entirely in PSUM - no scatter to HBM at all.
  - Per block: out[n, o] = dinv[n] * (agg^T @ W)[n, o] + b[o], DMA'd straight
    to the core's output slice.  (The linear layer commutes with the
    aggregation, so the GEMM runs on the 6.3k aggregated rows per core
    instead of all 50k input rows.)

The edge tables are padded so the instruction stream is identical on all 8
cores (run_bass_kernel_spmd compiles one program); only tensor data differs.
"""

import sys

sys.path.insert(0, "/opt/trn_rl_repo")

import numpy as np

import concourse.bass as bass
import concourse.bacc as bacc
import concourse.mybir as mybir
import concourse.tile as tile
import concourse.tile_sem_assignment as _tsa
from concourse.tile import add_dep_helper

# Tile round-robins SWDGE DMAs over the 8 DMASW sem lanes in scheduling
# order, which lets one sem serve instructions on different SWDGE queues.
# The ucode's per-queue ring reclaim then sees foreign increments (CoreSim
# flags this as "sem locked to SWDGE queue").  Pin lanes per queue instead:
# queue q only ever uses lanes {q, q+4}.
if not getattr(_tsa.TileClockTick, "_gcn_queue_aware", False):
    _orig_assign_tick = _tsa.TileClockTick._assign_tick

    def _assign_tick_queue_aware(self, inst):
        if (
            isinstance(inst, _tsa.DMAInst)
            and inst.engine == mybir.EngineType.Pool
            and not isinstance(inst, _tsa.bass_isa.UserSyncedRemoteDMADescs)
            and self.swdge_sem_count == _tsa.NUM_SWDGE_GLOBAL_SEMS
        ):
            q = getattr(inst, "queue_num", 0) or 0
            toggles = getattr(self, "_gcn_q_toggle", None)
            if toggles is None:
                toggles = self._gcn_q_toggle = [0, 0, 0, 0]
            self.next_sw_dma_idx = q + 4 * toggles[q]
            toggles[q] ^= 1
        return _orig_assign_tick(self, inst)

    _tsa.TileClockTick._assign_tick = _assign_tick_queue_aware
    _tsa.TileClockTick._gcn_queue_aware = True

N = 50000
E = 800000
F = 128          # in/out channels
P = 128
NCORES = 8
NB = 392         # node blocks incl. padding (= 8 * 49)
G = NB // NCORES  # 49 slots per core
LO = 32768       # gather-table split (int16 index limit)
NPAD = 51200     # padded node rows (= 25 chunks of 2048)
CHUNK = 2048
NCHUNKS = NPAD // CHUNK
QN = CHUNK // P  # node blocks per phase-A chunk
SPQ = 7          # slots per gather chunk
NQ = G // SPQ    # 7 gather chunks

f32 = mybir.dt.float32
fp16 = mybir.dt.float16
i32 = mybir.dt.int32
i16 = mybir.dt.int16

# hi gathers issued ahead of consumption (fills the GPSIMD window while the
# lo table half is still being built); the rest interleave per slot.
PRE_HI = 16


def _gather_order(T_LO, T_HI):
    """(side, slot) issue order shared by host packing and device build."""
    order = [("hi", g) for g in range(PRE_HI) if T_HI[g]]
    for g in range(G):
        if g >= PRE_HI and T_HI[g]:
            order.append(("hi", g))
        if T_LO[g]:
            order.append(("lo", g))
    return order


def _host_prep(x, W, b, edge_index):
    """Pure index manipulation + data staging; no FP math on the inputs."""
    x = np.asarray(x, dtype=np.float32)
    W = np.asarray(W, dtype=np.float32)
    b = np.asarray(b, dtype=np.float32)
    ei = np.asarray(edge_index)
    src = ei[0].astype(np.int64)
    dst = ei[1].astype(np.int64)

    cnt = np.bincount(dst, minlength=NPAD).astype(np.int64)

    # Sort edges by (dst block, src table half).
    ishi = (src >= LO).astype(np.int64)
    blk = dst >> 7
    order = np.lexsort((ishi, blk))
    src_s, dst_s, ishi_s, blk_s = src[order], dst[order], ishi[order], blk[order]
    bounds = np.searchsorted(blk_s, np.arange(NB + 1))

    # Per (core, slot) edge lists.  block = 8*g + c.
    lo_idx = [[None] * G for _ in range(NCORES)]
    lo_dst = [[None] * G for _ in range(NCORES)]
    hi_idx = [[None] * G for _ in range(NCORES)]
    hi_dst = [[None] * G for _ in range(NCORES)]
    for g in range(G):
        for c in range(NCORES):
            bb_ = 8 * g + c
            s0, s1 = bounds[bb_], bounds[bb_ + 1]
            mid = s0 + int(np.searchsorted(ishi_s[s0:s1], 1))
            sl = np.arange(128 * bb_, min(128 * (bb_ + 1), N), dtype=np.int64)
            li, ld = src_s[s0:mid], dst_s[s0:mid] - 128 * bb_
            hi, hd = src_s[mid:s1] - LO, dst_s[mid:s1] - 128 * bb_
            if bb_ < LO // 128:  # self loops go in the lo half
                li = np.concatenate([li, sl])
                ld = np.concatenate([ld, sl - 128 * bb_])
            else:
                hi = np.concatenate([hi, sl - LO])
                hd = np.concatenate([hd, sl - 128 * bb_])
            lo_idx[c][g], lo_dst[c][g] = li, ld
            hi_idx[c][g], hi_dst[c][g] = hi, hd

    # Shared tile counts (max over cores) keep the instruction stream uniform.
    T_LO = [max(1, max(-(-len(lo_idx[c][g]) // 128) for c in range(NCORES)))
            for g in range(G)]
    T_HI = [max(-(-len(hi_idx[c][g]) // 128) for c in range(NCORES))
            for g in range(G)]
    NT = sum(T_LO) + sum(T_HI)
    LTOT = NT * 8

    # The gather table is fp16; staging x as fp16 is numerically equivalent
    # (one extra rounding) and halves the phase-A HBM read.
    x_pad = np.zeros((NPAD, F), np.float16)
    x_pad[:N] = x.astype(np.float16)
    cnt_all = cnt.reshape(NPAD // P, P).T.astype(np.int32).copy()  # [128, 400]
    bb_host = np.tile(b[None, :], (P, 1)).astype(np.float32)
    iota_host = np.tile(np.arange(P, dtype=np.float16)[None, :], (P, 1)).copy()

    in_maps = []
    for c in range(NCORES):
        dstloc = np.full((P, NT), -1.0, np.float16)
        idx16 = np.zeros((P, LTOT), np.int16)
        # Packing order mirrors the device's gather issue order.
        col = icol = 0
        for side, g in _gather_order(T_LO, T_HI):
            if side == "lo":
                nt, li, ld = T_LO[g], lo_idx[c][g], lo_dst[c][g]
            else:
                nt, li, ld = T_HI[g], hi_idx[c][g], hi_dst[c][g]
            pi = np.zeros(nt * 128, np.int64)
            pi[: len(li)] = li
            pd = np.full(nt * 128, -1.0, np.float32)
            pd[: len(ld)] = ld
            dstloc[:, col : col + nt] = pd.reshape(nt, 128).T
            col += nt
            k8 = nt * 8
            idx16[:, icol : icol + k8] = np.tile(
                pi.reshape(-1, 16).T.astype(np.int16), (8, 1)
            )
            icol += k8
        cnt_slot = cnt_all[:, [8 * g + c for g in range(G)]].copy()
        in_maps.append(
            {
                "x": x_pad,
                "cnt_all": cnt_all,
                "cnt_slot": cnt_slot,
                "w": W,
                "bb": bb_host,
                "iota": iota_host,
                "dstloc": dstloc,
                "idx16": idx16,
            }
        )
    return in_maps, T_LO, T_HI


def build_nc(T_LO, T_HI, debug=False):
    NT = sum(T_LO) + sum(T_HI)
    LTOT = NT * 8
    nc = bacc.Bacc(
        "TRN2", target_bir_lowering=False, debug=debug, num_swdge_queues=4
    )

    x_d = nc.dram_tensor("x", [NPAD, F], fp16, kind="ExternalInput")
    cnt_all_d = nc.dram_tensor("cnt_all", [P, NPAD // P], i32, kind="ExternalInput")
    cnt_slot_d = nc.dram_tensor("cnt_slot", [P, G], i32, kind="ExternalInput")
    w_d = nc.dram_tensor("w", [F, F], f32, kind="ExternalInput")
    bb_d = nc.dram_tensor("bb", [P, F], f32, kind="ExternalInput")
    iota_d = nc.dram_tensor("iota", [P, P], fp16, kind="ExternalInput")
    dstloc_d = nc.dram_tensor("dstloc", [P, NT], fp16, kind="ExternalInput")
    idx_d = nc.dram_tensor("idx16", [P, LTOT], i16, kind="ExternalInput")
    out_d = nc.dram_tensor("out", [G * P, F], f32, kind="ExternalOutput")
    yp_d = nc.dram_tensor("yp", [NPAD, F], fp16)

    with tile.TileContext(nc) as tc:
        with (
            tc.tile_pool(name="const", bufs=1) as cp,
            tc.tile_pool(name="pa", bufs=3) as pa,
            tc.tile_pool(name="mlo", bufs=4) as plo,
            tc.tile_pool(name="mhi", bufs=1) as phi,  # one slot per per-slot tag
            tc.tile_pool(name="sel", bufs=5) as psel,
            tc.tile_pool(name="tt", bufs=3) as ptt,
            tc.tile_pool(name="osb", bufs=3) as posb,
            tc.tile_pool(name="agg", bufs=3, space="PSUM") as pagg,
            tc.tile_pool(name="gem", bufs=2, space="PSUM") as pgem,
        ):
            w_sb = cp.tile([F, F], f32)
            nc.sync.dma_start(out=w_sb[:], in_=w_d[:])
            bb_sb = cp.tile([P, F], f32)
            nc.sync.dma_start(out=bb_sb[:], in_=bb_d[:])
            iota_sb = cp.tile([P, P], fp16)
            nc.sync.dma_start(out=iota_sb[:], in_=iota_d[:])
            dstloc_sb = cp.tile([P, NT], fp16)
            nc.sync.dma_start(out=dstloc_sb[:], in_=dstloc_d[:])
            idx_sb = cp.tile([P, LTOT], i16)
            nc.sync.dma_start(out=idx_sb[:], in_=idx_d[:])

            # dinv = 1/sqrt(cnt + 1): full node grid + per-core slot grid
            def make_dinv(src_dram, ncols):
                t_i = cp.tile([P, ncols], i32)
                nc.sync.dma_start(out=t_i[:], in_=src_dram[:])
                t_f = cp.tile([P, ncols], f32)
                nc.vector.tensor_copy(out=t_f[:], in_=t_i[:])
                nc.scalar.activation(
                    out=t_f[:], in_=t_f[:],
                    func=mybir.ActivationFunctionType.Sqrt, bias=1.0, scale=1.0,
                )
                t_r = cp.tile([P, ncols], f32)
                nc.vector.reciprocal(out=t_r[:], in_=t_f[:])
                return t_r

            dinv_all = make_dinv(cnt_all_d, NPAD // P)
            dinv_slot = make_dinv(cnt_slot_d, G)

            # ---- phase A: yp = dinv * x, cast to fp16 ----
            # chunk k viewed as [p, q, f]: node = k*CHUNK + q*P + p
            def chunk_ap(dram, k):
                a = dram[:]
                return bass.AP(a.tensor, k * CHUNK * F, [[F, P], [P * F, QN], [1, F]])

            # hi-half chunks first: the hi gathers (35% of descriptor work)
            # can then start while the lo table half is still being built.
            n_lo_chunks = LO // CHUNK
            y_writes = [None] * NCHUNKS
            for k in list(range(n_lo_chunks, NCHUNKS)) + list(range(n_lo_chunks)):
                xt = pa.tile([P, CHUNK], fp16, tag="xt")
                nc.sync.dma_start(out=xt[:], in_=chunk_ap(x_d, k))
                yt = pa.tile([P, CHUNK], fp16, tag="yt")
                nc.vector.tensor_tensor(
                    out=yt[:].rearrange("p (q f) -> p q f", f=F),
                    in0=xt[:].rearrange("p (q f) -> p q f", f=F),
                    in1=dinv_all[:, QN * k : QN * (k + 1)].to_broadcast([P, QN, F]),
                    op=mybir.AluOpType.mult,
                )
                y_writes[k] = nc.sync.dma_start(out=chunk_ap(yp_d, k), in_=yt[:])

            # ---- phase B ----
            # One gather per (slot, table-half): independent destination
            # tiles let the 4 SWDGE queues generate descriptors concurrently
            # (HW-probed ~4x; same-tile slices would serialize under Tile).
            lo_tab = yp_d[0:LO, :]
            hi_tab = yp_d[LO:NPAD, :]
            lo_deps = y_writes[:n_lo_chunks]
            hi_deps = y_writes[n_lo_chunks:]
            col = icol = 0
            qrr = 0

            def gather(pool, tag, tab, nt, deps):
                nonlocal icol, qrr
                m = pool.tile([P, nt * F], fp16, tag=tag)
                gi = nc.gpsimd.dma_gather(
                    out_ap=m[:].rearrange("p (k f) -> p k f", f=F),
                    in_ap=tab,
                    idxs_ap=idx_sb[:, icol : icol + nt * 8],
                    num_idxs=nt * P,
                    num_idxs_reg=nt * P,
                    elem_size=F,
                    single_packet=False,
                    queue_num=qrr % 4,
                )
                qrr += 1
                for yw in deps:
                    add_dep_helper(gi.ins, yw.ins)
                icol += nt * 8
                return m

            tiles = {}
            cols = {}
            consumed_upto = [-1]

            def issue(side, g):
                nt = T_LO[g] if side == "lo" else T_HI[g]
                if side == "lo":
                    tiles[(side, g)] = gather(plo, "mlo", lo_tab, nt, lo_deps)
                else:
                    tiles[(side, g)] = gather(phi, f"mhi{g}", hi_tab, nt, hi_deps)
                cols[(side, g)] = col_cursor[0]
                col_cursor[0] += nt

            col_cursor = [0]
            order = _gather_order(T_LO, T_HI)
            oi = 0
            # pre-issue the head of the order up to the first lo gather's slot
            while oi < len(order) and order[oi][0] == "hi":
                issue(*order[oi])
                oi += 1

            for g in range(G):
                # issue this slot's remaining gathers (hi for g >= PRE_HI, lo)
                while oi < len(order) and (
                    order[oi][1] <= g
                    or (order[oi][0] == "hi" and order[oi][1] <= g + 1)
                ):
                    issue(*order[oi])
                    oi += 1
                ntl, nth = T_LO[g], T_HI[g]
                ntot = ntl + nth
                mlo = tiles.get(("lo", g))
                mhi = tiles.get(("hi", g))
                lo_col = cols.get(("lo", g), 0)
                hi_col = cols.get(("hi", g), 0)

                agg = pagg.tile([P, P], f32, tag="agg")
                mm = 0
                for (nt, m, base_col) in ((ntl, mlo, lo_col), (nth, mhi, hi_col)):
                    if nt == 0:
                        continue
                    S = psel.tile([P, nt * P], fp16, tag="S")
                    dl = dstloc_sb[:, base_col : base_col + nt]
                    nc.vector.tensor_tensor(
                        out=S[:].rearrange("p (t j) -> p t j", j=P),
                        in0=dl.to_broadcast([P, nt, P]),
                        in1=bass.AP(
                            iota_sb[:].tensor,
                            iota_sb[:].offset,
                            [iota_sb[:].ap[0], [0, nt], [1, P]],
                        ),
                        op=mybir.AluOpType.is_equal,
                    )
                    for t in range(nt):
                        nc.tensor.matmul(
                            out=agg[:],
                            lhsT=m[:, t * F : (t + 1) * F],
                            rhs=S[:, t * P : (t + 1) * P],
                            start=(mm == 0),
                            stop=(mm == ntot - 1),
                        )
                        mm += 1

                if True:
                    tt = ptt.tile([P, P], f32, tag="tt")
                    nc.scalar.activation(
                        out=tt[:], in_=agg[:],
                        func=mybir.ActivationFunctionType.Copy,
                    )
                    gem = pgem.tile([P, P], f32, tag="gem")
                    nc.tensor.matmul(
                        out=gem[:], lhsT=tt[:], rhs=w_sb[:], start=True, stop=True
                    )
                    osb = posb.tile([P, P], f32, tag="osb")
                    nc.vector.tensor_scalar(
                        out=osb[:], in0=gem[:],
                        scalar1=dinv_slot[:, g : g + 1], scalar2=None,
                        op0=mybir.AluOpType.mult,
                    )
                    nc.vector.tensor_tensor(
                        out=osb[:], in0=osb[:], in1=bb_sb[:],
                        op=mybir.AluOpType.add,
                    )
                    nc.sync.dma_start(
                        out=out_d[g * P : (g + 1) * P, :], in_=osb[:]
                    )

    nc.compile()
    return nc


def _assemble(results):
    out = np.zeros((NB * P, F), np.float32)
    for c in range(NCORES):
        oc = results[c]["out"]
        for g in range(G):
            out[(8 * g + c) * P : (8 * g + c + 1) * P] = oc[g * P : (g + 1) * P]
    return out[:N]


def kernel(x, W, b, edge_index):
    from concourse.bass_utils import run_bass_kernel_spmd

    in_maps, T_LO, T_HI = _host_prep(x, W, b, edge_index)
    nc = build_nc(T_LO, T_HI)
    res = run_bass_kernel_spmd(nc, in_maps, list(range(NCORES)))
    return _assemble(res.results)
